# revision 36
# baseline (speedup 1.0000x reference)
"""Trainium2 Bass kernel for nn_AdapterMLP (gnn_message_passing).

Strategy (8 independent NeuronCores, no collectives):
  - Shard (batch=4) x (seq halves=2) -> 8 shards of [1024, 4096] rows.
  - All gather/scatter index structure is resolved on the host into
    dense one-hot matmul operands (A_g for the subtoken mean-pool
    gather, S_sel for the last-wins scatter); the device kernel is
    pure dense compute.
  - Main MLP x @ (mlp_w[:, :D]*ln).T runs in fp8e4 DoubleRow (2
    k-tiles per PE instruction): x scaled by S_X, weights by S_W, the
    combined scale SC folded out through the per-row RMS scale.
  - Algebraic shortcut: aw[w,e] = <(g*u)[w,e,:], (b @ down_w)[w,:]>,
    eliminating the per-item down-projection.
  - Single paced loop: 64 matmul groups; the first L_PARK groups park
    PSUM->SBUF (bf16) and defer their epilogue until the word branch
    delivers aux/scal; later groups fuse the scatter k-tile into PSUM
    and run the silu epilogue directly.  Word-branch work is placed
    into fixed slots of the loop so no engine queue head-blocks.
"""
import os
import sys

sys.path.insert(0, "/opt/trn_rl_repo")

import numpy as np
from ml_dtypes import bfloat16, float8_e4m3

import concourse.bass as bass
import concourse.bacc as bacc
import concourse.tile as tile
from concourse import mybir
from concourse.bass_utils import run_bass_kernel_spmd

B, S, D = 4, 2048, 4096
W, E, T = 128, 8, 4
KD, KI = 100, 1024
EPS = 1e-06
NCORES = 8
SL = S // 2        # 1024 rows per core
GR = 512           # gathered rows per core (W*T upper bound)
P = 128
FB = 512           # psum free dim
NK = D // P        # 32 k-tiles
NN = D // FB       # 8 n-chunks
NM = SL // P       # 8 m-tiles
NE = E + 1         # 9

f32 = mybir.dt.float32
bf = mybir.dt.bfloat16
f8 = mybir.dt.float8e4
AF = mybir.ActivationFunctionType
ALU = mybir.AluOpType
AX = mybir.AxisListType
PM = mybir.MatmulPerfMode

S_X = 16.0       # fp8 scale on activations (x)
S_W = 512.0      # fp8 scale on weights
SC = S_X * S_W   # folded out via the per-row RMS scale

L_PARK = 16      # groups parked before the word branch lands


def _bf(a):
    return np.ascontiguousarray(a.astype(bfloat16))


def _f8(a):
    return np.ascontiguousarray(np.clip(a, -240.0, 240.0).astype(float8_e4m3))


def build_core_inputs(inp, core):
    """Host preprocessing for one core: slice/transpose/cast + index->matrix."""
    b, h = core // 2, core % 2
    r0 = h * SL
    x = np.asarray(inp["output_hidden_states"], np.float32)
    we_b = np.asarray(inp["words_ents"])[b]
    ws_b = np.asarray(inp["words_subtoken"])[b]
    ce = np.asarray(inp["concept_embed"], np.float32)
    sent = np.asarray(inp["sentinel"], np.float32).reshape(KD)
    lnw = np.asarray(inp["ln_weight"], np.float32)
    gw = np.asarray(inp["gate_w"], np.float32)
    uw = np.asarray(inp["up_w"], np.float32)
    dw = np.asarray(inp["down_w"], np.float32)
    mw = np.asarray(inp["mlp_w"], np.float32)
    mb = np.asarray(inp["mlp_b"], np.float32)
    alpha = float(np.asarray(inp["alpha"]).reshape(-1)[0])

    xl = x[b, r0:r0 + SL]                                    # [SL, D]
    xt = np.ascontiguousarray(xl.T)                          # [D, SL]

    # b-gather rows: unique subtoken indices of this item (pad index S dropped)
    idxf = np.where(ws_b == -1, S, ws_b)                     # [W,T]
    flat = idxf.reshape(-1)
    uniq = np.unique(flat[flat < S])
    gidx = np.zeros(GR, np.int64)
    gidx[:uniq.size] = uniq
    xg = x[b, gidx]                                          # [GR, D]
    cnt = np.maximum(np.sum(ws_b != -1, axis=1), 1).astype(np.float32)
    # per-gathered-row RMS scale folded into the gather matrix
    rc_g = 1.0 / np.sqrt(np.mean(xg * xg, axis=1) + EPS)     # [GR]
    ag = np.zeros((GR, W), np.float32)
    pos = {int(s_): i for i, s_ in enumerate(uniq)}
    for w in range(W):
        for t in range(T):
            s_ = int(idxf[w, t])
            if s_ < S:
                ag[pos[s_], w] += rc_g[pos[s_]] / cnt[w]

    # entity embeddings (host gather of the concept table)
    we_idx = np.where(we_b == -1, 0, we_b)
    ents = ce[we_idx]                                        # [W,E,KD]
    ent_ori = np.concatenate(
        [ents, np.broadcast_to(sent.reshape(1, 1, KD), (W, 1, KD))], axis=1)
    entw = np.ascontiguousarray(ent_ori.transpose(1, 0, 2))  # [NE, W, KD]
    entt = np.zeros((P, NE * W), np.float32)                 # KD padded to 128
    entt[:KD] = entw.reshape(NE * W, KD).T

    # scatter one-hot: winner = last (w,t) in flat order; local half only
    sst = np.zeros((W, SL), np.float32)
    winner = {}
    for w in range(W):
        for t in range(T):
            s_ = int(idxf[w, t])
            if s_ < S:
                winner[s_] = w
    for s_, w in winner.items():
        if r0 <= s_ < r0 + SL:
            sst[w, s_ - r0] = 1.0

    # weights: fold ln into Wh and down_w; pre-transpose; tile wk for DMA
    whT = (mw[:, :D] * lnw[None, :]).T                       # [D, D]
    wtT = mw[:, D:].T                                        # [KD, D]
    wk = np.zeros((NN, NK + 1, P, FB), np.float32)
    for n in range(NN):
        cs = slice(n * FB, (n + 1) * FB)
        for k in range(NK):
            wk[n, k] = whT[k * P:(k + 1) * P, cs]
        wk[n, NK, :KD] = wtT[:, cs]
        wk[n, NK, KD] = mb[cs]
    dwt = dw * lnw[:, None]                                  # [D, KI]

    mask = np.where(
        np.concatenate([we_b, np.ones((W, 1), we_b.dtype)], -1) == -1,
        -1e9, 0.0).astype(np.float32)

    aux_init = np.zeros((P, SL), np.float32)
    aux_init[KD] = 1.0

    # per-row RMS statistics (host-side, like the gather-matrix scales)
    sd_row = np.sqrt(np.mean(xl * xl, axis=1) + EPS)         # [SL]
    binv_h = np.broadcast_to((SC * sd_row)[None, :], (P, SL))
    scal_h = (1.0 / (SC * sd_row)).reshape(NM, P).T          # [P, NM]

    # batch 4 k-tiles per DMA: [G, 128, 4*inner] contiguous blocks
    xt_big = xt.reshape(NK, P, SL).reshape(8, 4, P, SL).transpose(0, 2, 1, 3).reshape(8, P, 4 * SL)
    wk_big = wk[:, :NK].reshape(NN, 8, 4, P, FB).transpose(0, 1, 3, 2, 4).reshape(NN, 8, P, 4 * FB)
    wk_aux = np.ascontiguousarray(wk[:, NK])
    dwt_big = dwt.reshape(NK, P, KI).reshape(16, 2, P, KI).transpose(0, 2, 1, 3).reshape(16, P, 2 * KI)
    return dict(
        xt=_f8(xt_big * S_X),
        xrow_bf=_bf(xl),
        wk=_f8(wk_big * S_W),
        wk_aux=_bf(wk_aux),
        dwt=_f8(dwt_big * S_W),
        xg=_f8(xg * S_X).reshape(4, P, D),
        ag=_bf(ag).reshape(4, P, W),
        entw=_bf(entw),
        entt=_bf(entt),
        gwt=_bf(np.concatenate([gw.T, np.zeros((P - KD, KI), np.float32)], 0)),
        uwt=_bf(np.concatenate([uw.T, np.zeros((P - KD, KI), np.float32)], 0)),
        sst=_f8(sst),
        mask=np.ascontiguousarray(mask),
        alpha_b=np.full((P, 1), alpha, np.float32),
        aux_init=_bf(aux_init),
        binv=_bf(binv_h),
        scal=np.ascontiguousarray(scal_h),
    )


def _kernel_body(nc, tc, I, out_ap):
    res = tc.alloc_tile_pool(name="res", bufs=1)
    small = tc.alloc_tile_pool(name="small", bufs=1)
    parkp = tc.alloc_tile_pool(name="park", bufs=1)
    wkp = tc.alloc_tile_pool(name="wkp", bufs=1)
    wkap = tc.alloc_tile_pool(name="wka", bufs=1)
    wp = tc.alloc_tile_pool(name="word", bufs=1)
    xgp = tc.alloc_tile_pool(name="xgp", bufs=1)
    mps = tc.alloc_tile_pool(name="mpsum", bufs=1, space="PSUM")
    wps = tc.alloc_tile_pool(name="wpsum", bufs=1, space="PSUM")
    gps = tc.alloc_tile_pool(name="gpsum", bufs=1, space="PSUM")

    # ======== sync-queue DMAs: xt groups, then xrow_bf ========
    gwt_t = wp.tile([P, KI], bf, tag="gwt")
    nc.sync.dma_start(out=gwt_t[:], in_=I["gwt"][:])
    uwt_t = wp.tile([P, KI], bf, tag="uwt")
    nc.sync.dma_start(out=uwt_t[:], in_=I["uwt"][:])
    entt_t = wp.tile([P, NE * W], bf, tag="entt")
    nc.sync.dma_start(out=entt_t[:], in_=I["entt"][:])
    ags_t = wp.tile([P, 4 * W], bf, tag="ags")
    for g in range(4):
        nc.sync.dma_start(out=ags_t[:, g * W:(g + 1) * W], in_=I["ag"][g])
    scal_t = small.tile([P, NM], f32, tag="scal")
    nc.sync.dma_start(out=scal_t[:], in_=I["scal"][:])
    xt_big = []
    for g in range(8):
        t = res.tile([P, 4 * SL], f8, tag=f"xt{g}", name=f"xtt{g}")
        xt_big.append(t)
    for g in range(8):
        nc.sync.dma_start(out=xt_big[g][:], in_=I["xt"][g])
    xg_tiles = []
    for g in range(4):
        xg_t = xgp.tile([P, D], f8, tag=f"xg{g}", name=f"xg{g}")
        nc.sync.dma_start(out=xg_t[:], in_=I["xg"][g])
        xg_tiles.append(xg_t)
    dwt_tiles = []
    for kb in range(16):
        dwt_t = wp.tile([P, 2 * KI], f8, tag=f"dwt{kb % 8}", name=f"dwt{kb}")
        nc.sync.dma_start(out=dwt_t[:], in_=I["dwt"][kb])
        dwt_tiles.append(dwt_t)

    def xt_pair(j, kk, m):
        # [P, 2, 128] stationary pair: k-tiles 4j+kk, 4j+kk+1
        return xt_big[j][:].rearrange(
            "p (k s) -> p k s", k=4)[:, kk:kk + 2, m * P:(m + 1) * P]

    def wk_pair(wt, kk):
        # [P, 2, FB] moving pair matching xt_pair's k-tiles
        return wt[:].rearrange("p (k f) -> p k f", k=4)[:, kk:kk + 2, :]

    # ======== scalar-queue DMAs: small word-branch inputs only ========
    binv_t = res.tile([P, SL], bf, tag="binv")
    nc.scalar.dma_start(out=binv_t[:], in_=I["binv"][:])
    ent_t = wp.tile([P, NE * KD], bf, tag="entw")
    for e in range(NE):
        nc.scalar.dma_start(out=ent_t[:, e * KD:(e + 1) * KD], in_=I["entw"][e])
    sst_t = wp.tile([P, SL], f8, tag="sst")
    nc.scalar.dma_start(out=sst_t[:], in_=I["sst"][:])
    mask_t = small.tile([P, NE], f32, tag="mask")
    nc.scalar.dma_start(out=mask_t[:], in_=I["mask"][:])
    alpha_t = small.tile([P, 1], f32, tag="alpha")
    nc.scalar.dma_start(out=alpha_t[:], in_=I["alpha_b"][:])
    aux_t = res.tile([P, SL], bf, tag="aux")
    nc.scalar.dma_start(out=aux_t[:], in_=I["aux_init"][:])

    # ======== gpsimd-queue DMAs: wk chunks + wk_aux ========
    wk_cache = {}

    def wk_chunk(n):
        if n in wk_cache:
            return wk_cache[n]
        grp = []
        for j in range(8):
            wt = wkp.tile([P, 4 * FB], f8, tag=f"wkg{j}", bufs=2,
                          name=f"wk{n}g{j}")
            nc.gpsimd.dma_start(out=wt[:], in_=I["wk"][n, j])
            grp.append(wt)
        wk_cache[n] = grp
        return grp

    wk_chunk(0)
    wk_chunk(1)
    wka_tiles = []
    for n in range(NN):
        wa = wkap.tile([P, FB], bf, tag=f"wka{n}")
        nc.gpsimd.dma_start(out=wa[:], in_=I["wk_aux"][n])
        wka_tiles.append(wa)


    # ======== word-branch state (filled in by slot hooks) ========
    st = {}

    def hook_bt(half):
        if half == 0:
            st["bt"] = wp.tile([P, NK * W], f8, tag="btall", name="btall")
        bt_all = st["bt"]
        for dk in range(half * 16, half * 16 + 16):
            ps = wps.tile([P, W], f32, tag="wps", bufs=2, name=f"btps{dk}")
            for g in range(4):
                nc.tensor.matmul(ps[:],
                                 lhsT=xg_tiles[g][:, dk * P:(dk + 1) * P],
                                 rhs=ags_t[:, g * W:(g + 1) * W],
                                 start=(g == 0), stop=(g == 3))
            nc.scalar.copy(bt_all[:, dk * W:(dk + 1) * W], ps[:])

    def hook_c(half):
        # c = b @ (down_w * lnw) in fp8 DoubleRow; bt*S_X, dwt*S_W -> /SC
        if half == 0:
            cps = []
            for i2 in range(2):
                cpsi = wps.tile([P, FB], f32, tag="wps", bufs=2,
                                name=f"c_ps{i2}")
                cps.append(cpsi)
            st["cps"] = cps
        cps = st["cps"]
        bt_pairs = st["bt"][:].rearrange("p (k w) -> p k w", k=NK)
        for kb in range(half * 8, half * 8 + 8):
            dw_pairs = dwt_tiles[kb][:].rearrange("p (kk f) -> p kk f", kk=2)
            for i2 in range(2):
                nc.tensor.matmul(
                    cps[i2][:], lhsT=bt_pairs[:, 2 * kb:2 * kb + 2, :],
                    rhs=dw_pairs[:, :, i2 * FB:(i2 + 1) * FB],
                    start=(kb == 0), stop=(kb == 15),
                    perf_mode=PM.DoubleRow)
        if half == 1:
            c_bf = wp.tile([P, KI], bf, tag="c")
            for i2 in range(2):
                nc.scalar.activation(c_bf[:, i2 * FB:(i2 + 1) * FB],
                                     cps[i2][:], AF.Copy, scale=1.0 / SC)
            st["c"] = c_bf

    def hook_gateup(e):
        g_sb = wp.tile([P, KI], bf, tag="gsb", bufs=3, name=f"gsb{e}")
        gu = wp.tile([P, KI], bf, tag="gu", bufs=9, name=f"gu{e}")
        for i2 in range(2):
            gp = gps.tile([P, FB], f32, tag="gps", bufs=2, name=f"gp{e}_{i2}")
            nc.tensor.matmul(gp[:], lhsT=entt_t[:, e * P:(e + 1) * P],
                             rhs=gwt_t[:, i2 * FB:(i2 + 1) * FB],
                             start=True, stop=True)
            nc.scalar.activation(g_sb[:, i2 * FB:(i2 + 1) * FB], gp[:], AF.Silu)
            up = gps.tile([P, FB], f32, tag="gps", bufs=2, name=f"up{e}_{i2}")
            nc.tensor.matmul(up[:], lhsT=entt_t[:, e * P:(e + 1) * P],
                             rhs=uwt_t[:, i2 * FB:(i2 + 1) * FB],
                             start=True, stop=True)
            nc.vector.tensor_mul(gu[:, i2 * FB:(i2 + 1) * FB],
                                 g_sb[:, i2 * FB:(i2 + 1) * FB], up[:])
        st[f"gu{e}"] = gu

    def hook_aw(e):
        if "aw" not in st:
            st["aw"] = small.tile([P, NE], f32, tag="aw", name="aw_t")
        scr = wp.tile([P, KI], bf, tag="awscr", bufs=2, name=f"awscr{e}")
        nc.vector.tensor_mul(scr[:], st[f"gu{e}"][:], st["c"][:])
        scr2 = wp.tile([P, KI], bf, tag="awscr2", bufs=2, name=f"awscr2_{e}")
        nc.scalar.activation(scr2[:], scr[:], AF.Copy,
                             accum_out=st["aw"][:, e:e + 1])

    def hook_softmax(_i):
        aw_t = st["aw"]
        awm = small.tile([P, NE], f32, tag="awm")
        nc.vector.tensor_add(awm[:], aw_t[:], mask_t[:])
        mx = small.tile([P, 1], f32, tag="mx")
        nc.vector.reduce_max(mx[:], awm[:], axis=AX.X)
        nmx = small.tile([P, 1], f32, tag="nmx")
        nc.vector.tensor_scalar_mul(nmx[:], mx[:], -1.0)
        expt = small.tile([P, NE], f32, tag="expt")
        sume = small.tile([P, 1], f32, tag="sume")
        nc.scalar.activation(expt[:], awm[:], AF.Exp, bias=nmx[:],
                             accum_out=sume[:])
        rse = small.tile([P, 1], f32, tag="rse")
        nc.vector.reciprocal(rse[:], sume[:])
        attn = small.tile([P, NE], f32, tag="attn")
        nc.vector.tensor_scalar_mul(attn[:], expt[:], rse[:])
        acc_prev = wp.tile([P, KD], f32, tag="acc", bufs=2)
        nc.vector.tensor_scalar_mul(acc_prev[:], ent_t[:, 0:KD], attn[:, 0:1])
        for e in range(1, NE):
            acc_new = wp.tile([P, KD], f32, tag="acc", bufs=2, name=f"acc{e}")
            nc.vector.scalar_tensor_tensor(
                out=acc_new[:], in0=ent_t[:, e * KD:(e + 1) * KD],
                scalar=attn[:, e:e + 1], in1=acc_prev[:],
                op0=ALU.mult, op1=ALU.add)
            acc_prev = acc_new
        ao_pad = wp.tile([P, P], bf, tag="ao_pad")
        nc.vector.memset(ao_pad[:], 0.0)
        nc.scalar.copy(ao_pad[:, 0:KD], acc_prev[:])
        st["ao_pad"] = ao_pad

    def hook_scatter(_i):
        # scatter matmul into aux k-tile
        for i2 in range(SL // FB):
            tps = wps.tile([P, FB], f32, tag="wps", bufs=2, name=f"tps{i2}")
            nc.tensor.matmul(tps[:], lhsT=st["ao_pad"][:],
                             rhs=sst_t[:, i2 * FB:(i2 + 1) * FB],
                             start=True, stop=True)
            nc.scalar.copy(aux_t[0:KD, i2 * FB:(i2 + 1) * FB], tps[0:KD, :])
        nc.vector.tensor_mul(aux_t[:], aux_t[:], binv_t[:])

    def hook_release_xg(_i):
        xgp.release()

    def hook_release(_i):
        wp.release()
        gps.release()
        wps.release()

    # slot -> list of thunks, placed so inputs are ready when the queue
    # reaches them
    hooks = {
        0: [lambda: hook_gateup(0), lambda: hook_gateup(1),
            lambda: hook_gateup(2), lambda: hook_gateup(3),
            lambda: hook_gateup(4), lambda: hook_gateup(5),
            lambda: hook_gateup(6), lambda: hook_gateup(7),
            lambda: hook_gateup(8)],
        2: [lambda: hook_bt(0)],
        3: [lambda: hook_bt(1)],
        5: [lambda: hook_c(0), lambda: hook_c(1),
            lambda: hook_release_xg(0)],
        6: [lambda: hook_aw(0), lambda: hook_aw(1), lambda: hook_aw(2)],
        7: [lambda: hook_aw(3), lambda: hook_aw(4), lambda: hook_aw(5)],
        8: [lambda: hook_aw(6), lambda: hook_aw(7), lambda: hook_aw(8)],
        9: [lambda: hook_softmax(0)],
        10: [lambda: hook_scatter(0)],
        11: [lambda: hook_release(0)],
    }

    park_tiles = [None] * L_PARK
    eps_pool = [None]
    op_pool = [None]

    def op():
        if op_pool[0] is None:
            op_pool[0] = tc.alloc_tile_pool(name="op", bufs=1)
        return op_pool[0]

    def emit_fused_epilogue(n, m, pm):
        pre_sb = op().tile([P, FB], f32, tag="pre", bufs=4, name=f"pre{n}_{m}")
        nc.scalar.activation(pre_sb[:], pm[:], AF.Silu,
                             scale=scal_t[:, m:m + 1])
        xr_c = op().tile([P, FB], bf, tag="xrc", bufs=4, name=f"xrc{n}_{m}")
        nc.sync.dma_start(
            out=xr_c[:],
            in_=I["xrow_bf"][m * P:(m + 1) * P, n * FB:(n + 1) * FB])
        o_c = op().tile([P, FB], f32, tag="oc", bufs=4, name=f"oc{n}_{m}")
        nc.vector.scalar_tensor_tensor(
            out=o_c[:], in0=pre_sb[:], scalar=alpha_t[:], in1=xr_c[:],
            op0=ALU.mult, op1=ALU.add)
        nc.sync.dma_start(
            out=out_ap[m * P:(m + 1) * P, n * FB:(n + 1) * FB], in_=o_c[:])

    def emit_early_epilogue(idx):
        n, m = idx // NM, idx % NM
        if eps_pool[0] is None:
            eps_pool[0] = tc.alloc_tile_pool(name="epsum", bufs=1,
                                             space="PSUM")
        pa = eps_pool[0].tile([P, FB], f32, tag="pa", bufs=4, name=f"pa{idx}")
        nc.tensor.matmul(pa[:], lhsT=aux_t[:, m * P:(m + 1) * P],
                         rhs=wka_tiles[n][:], start=True, stop=True)
        pre_sb = op().tile([P, FB], f32, tag="pre", bufs=4, name=f"preE{idx}")
        nc.vector.tensor_add(pre_sb[:], park_tiles[idx][:], pa[:])
        nc.scalar.activation(pre_sb[:], pre_sb[:], AF.Silu,
                             scale=scal_t[:, m:m + 1])
        xr_c = op().tile([P, FB], bf, tag="xrc", bufs=4, name=f"xrcE{idx}")
        nc.sync.dma_start(
            out=xr_c[:],
            in_=I["xrow_bf"][m * P:(m + 1) * P, n * FB:(n + 1) * FB])
        o_c = op().tile([P, FB], f32, tag="oc", bufs=4, name=f"ocE{idx}")
        nc.vector.scalar_tensor_tensor(
            out=o_c[:], in0=pre_sb[:], scalar=alpha_t[:], in1=xr_c[:],
            op0=ALU.mult, op1=ALU.add)
        nc.sync.dma_start(
            out=out_ap[m * P:(m + 1) * P, n * FB:(n + 1) * FB], in_=o_c[:])

    # ======== the single paced loop ========
    ep_next = 0
    for i in range(NN * NM):
        n, m = i // NM, i % NM
        if m == 0 and n + 2 < NN:
            wk_chunk(n + 2)
        grp = wk_chunk(n)
        pm = mps.tile([P, FB], f32, tag="pm", bufs=4, name=f"pm{n}_{m}")
        for j in range(8):
            for kk in (0, 2):
                nc.tensor.matmul(
                    pm[:], lhsT=xt_pair(j, kk, m), rhs=wk_pair(grp[j], kk),
                    start=(j == 0 and kk == 0),
                    stop=(i < L_PARK and j == 7 and kk == 2),
                    perf_mode=PM.DoubleRow)
        if i < L_PARK:
            pk = parkp.tile([P, FB], bf, tag="park", bufs=L_PARK + 2,
                            name=f"park{i}")
            nc.scalar.copy(pk[:], pm[:])
            park_tiles[i] = pk
        else:
            nc.tensor.matmul(pm[:], lhsT=aux_t[:, m * P:(m + 1) * P],
                             rhs=wka_tiles[n][:], start=False, stop=True)
            emit_fused_epilogue(n, m, pm)
        for th in hooks.get(i, []):
            th()
        if i >= L_PARK and ep_next < L_PARK:
            emit_early_epilogue(ep_next)
            ep_next += 1
    while ep_next < L_PARK:
        emit_early_epilogue(ep_next)
        ep_next += 1
    if op_pool[0] is not None:
        op_pool[0].release()
    for pool in (wkap, wkp, parkp, small, res):
        pool.release()
    if eps_pool[0] is not None:
        eps_pool[0].release()
    mps.release()


_CACHE = {}


def _build():
    if "nc" in _CACHE:
        return _CACHE["nc"]
    nc = bacc.Bacc("TRN2", target_bir_lowering=False, debug=False,
                   num_devices=NCORES)
    shapes = dict(
        xt=([8, P, 4 * SL], f8), xrow_bf=([SL, D], bf),
        wk=([NN, 8, P, 4 * FB], f8), wk_aux=([NN, P, FB], bf),
        dwt=([16, P, 2 * KI], f8), xg=([4, P, D], f8), ag=([4, P, W], bf),
        entw=([NE, W, KD], bf), entt=([P, NE * W], bf), gwt=([P, KI], bf),
        uwt=([P, KI], bf), sst=([W, SL], f8), mask=([W, NE], f32),
        alpha_b=([P, 1], f32), aux_init=([P, SL], bf),
        binv=([P, SL], bf), scal=([P, NM], f32),
    )
    I = {name: nc.dram_tensor(name, shp, dt, kind="ExternalInput").ap()
         for name, (shp, dt) in shapes.items()}
    out_ap = nc.dram_tensor("out", [SL, D], f32, kind="ExternalOutput").ap()
    with tile.TileContext(nc) as tc:
        _kernel_body(nc, tc, I, out_ap)
    nc.compile()
    _CACHE["nc"] = nc
    return nc


def kernel(**inputs):
    nc = _build()
    in_maps = [build_core_inputs(inputs, c) for c in range(NCORES)]
    res = run_bass_kernel_spmd(nc, in_maps, core_ids=list(range(NCORES)))
    out = np.empty((B, S, D), np.float32)
    for c in range(NCORES):
        b, h = c // 2, c % 2
        out[b, h * SL:(h + 1) * SL] = res.results[c]["out"]
    return out


if __name__ == "__main__":
    rng = np.random.default_rng(0)
    inp = {
        "output_hidden_states": rng.standard_normal((B, S, D)).astype(np.float32),
        "words_ents": rng.integers(0, 100000, (B, W, E)).astype(np.int64),
        "words_subtoken": rng.integers(0, S, (B, W, T)).astype(np.int64),
        "input_ids": rng.integers(0, 32000, (B, S)).astype(np.int64),
        "concept_embed": (rng.standard_normal((100000, KD)) * 0.02).astype(np.float32),
        "sentinel": (rng.standard_normal((1, KD)) * 0.02).astype(np.float32),
        "ln_weight": np.ones(D, np.float32),
        "gate_w": (rng.standard_normal((KI, KD)) * 0.02).astype(np.float32),
        "up_w": (rng.standard_normal((KI, KD)) * 0.02).astype(np.float32),
        "down_w": (rng.standard_normal((D, KI)) * 0.02).astype(np.float32),
        "mlp_w": (rng.standard_normal((D, D + KD)) * 0.01).astype(np.float32),
        "mlp_b": np.zeros(D, np.float32),
        "alpha": np.array([0.5], np.float32),
    }
    out = kernel(**inp)
    print("kernel ran, out shape", out.shape, "mean", out.mean())
